# revision 2
# baseline (speedup 1.0000x reference)
"""Trainium2 Bass kernel for nn_MLPBuilder (GNN message-passing edge predictor).

Math: adj[i,j] = argmax_o softmax(W2 @ relu(W1 @ cat(x_i, x_j) + b1) + b2)
            = 1  iff  w . relu(la_i + lb_j + b1) + c > 0
  where la = x @ W1[:, :D].T, lb = x @ W1[:, D:].T,
        w = W2[1] - W2[0], c = b2[1] - b2[0]   (softmax+argmax == threshold).

Sharding: rows of the N^2 pair grid, 128 i-rows per core (8 cores).

Per core (all fp32):
 - lbT[hh][h', j]  [128, 1024]: lb transposed, h on partitions (hh = h-half)
 - labT[hh][h', i] [128, 128] : la + b1 transposed (per-partition relu bias)
 - relu tiles r = relu(lbT + labT[:, i]) via ScalarE (activation w/ bias) and
   VectorE (tensor_scalar add+max), single-writer tiles per 512-col chunk
 - h-reduction on PE: stationary [128, 128] = w_half in column 32c, zeros
   elsewhere -> psum row 32c accumulates the logit row for i = 4g+c.
   8 matmuls (4 i x 2 hh) accumulate into each psum bank [128, 512].
 - evacuation: ScalarE Sign(psum + c) -> uint8 (1 iff adj=1), DMA rows
   {0, 32, 64, 96} (partition stride 32) to DRAM.

Sync-wait budget: walrus allows ~1 sync wait on a matmul (LDWEIGHTS struct),
so inputs are packed into two DRAM tensors (inA for the lb pass, inB for the
rest) and op order ensures every instruction newly waits on at most one
semaphore.
"""

import numpy as np

import concourse.bass as bass
import concourse.bacc as bacc
import concourse.mybir as mybir
from concourse.tile import TileContext
from concourse.bass_utils import run_bass_kernel_spmd

N, D, H = 1024, 128, 256
NCORES = 8
RPC = N // NCORES  # 128 i-rows per core
FP32 = mybir.dt.float32
JA = 512  # ScalarE handles relu cols [0:JA) of h-half 0; VectorE the rest

# inA columns: [w1bT (256) | xT (1024)]
A_W1B, A_XT = 0, 256
# inB columns: [w1aT (256) | xiT (128) | wst (8*128) | b1c (2)]
B_W1A, B_XI, B_WST, B_B1C = 0, 256, 384, 1408

TRACE = False
LAST_RESULTS = None


def build_nc(cdiff: float, n_groups: int = RPC // 4):
    AF = mybir.ActivationFunctionType
    ALU = mybir.AluOpType

    nc = bacc.Bacc(None, target_bir_lowering=False)
    inA = nc.declare_dram_parameter("inA", [128, 1280], FP32, isOutput=False)
    inB = nc.declare_dram_parameter("inB", [128, 1410], FP32, isOutput=False)
    adj8 = nc.declare_dram_parameter("adj8", [RPC, N], mybir.dt.uint8, isOutput=True)

    with TileContext(nc) as tc:
        with (
            tc.tile_pool(name="const", bufs=1) as cpool,
            tc.tile_pool(name="relu", bufs=3) as rpool,
            tc.tile_pool(name="adj", bufs=3) as apool,
            tc.tile_pool(name="mm", bufs=2, space="PSUM") as mmpool,
            tc.tile_pool(name="setup_ps", bufs=2, space="PSUM") as spool,
            tc.tile_pool(name="setup_ps2", bufs=1, space="PSUM") as spool2,
            tc.tile_pool(name="dummy_ps", bufs=1, space="PSUM") as dpool,
        ):
            inA_sb = cpool.tile([128, 1280], FP32)
            # chunk 0 carries w1bT + xT[:, :512]; chunk 1 the rest of xT
            nc.sync.dma_start(out=inA_sb[:, :768], in_=inA[:, :768])
            nc.sync.dma_start(out=inA_sb[:, 768:], in_=inA[:, 768:])
            inB_sb = cpool.tile([128, 1410], FP32)
            nc.sync.dma_start(out=inB_sb[:], in_=inB[:])

            w1bT_sb = inA_sb[:, A_W1B : A_W1B + 256]
            xT_sb = inA_sb[:, A_XT : A_XT + 1024]
            w1aT_sb = inB_sb[:, B_W1A : B_W1A + 256]
            xiT_sb = inB_sb[:, B_XI : B_XI + 128]
            b1c_sb = inB_sb[:, B_B1C : B_B1C + 2]

            def wst_sb(c, hh):
                o = B_WST + (2 * c + hh) * 128
                return inB_sb[:, o : o + 128]

            # cbias: [128,1] = cdiff, for the Sign evacuation
            cbias = cpool.tile([128, 1], FP32)
            nc.vector.memset(cbias[:], cdiff)
            # ScalarE pre-touch of inB so later ACT ops never add a DMA wait
            sct = cpool.tile([128, 1], FP32)
            nc.scalar.copy(sct[:], inB_sb[:, B_B1C : B_B1C + 1])

            # ---- lbT[hh] = (x @ W1b.T).T, h on partitions ----
            lbT = []
            for hh in range(2):
                t = cpool.tile([128, N], FP32, tag=f"lbT{hh}", name=f"lbT{hh}")
                lbT.append(t)
            for jc in range(2):  # jc outer: chunk-0 DMA gates jc=0 MMs only
                if jc == 1:
                    # wait-collector: absorb the chunk-1 DMA wait on PE so the
                    # real jc=1 matmuls carry only their PSUM-WAR wait
                    dps = dpool.tile([1, 1], FP32, tag="dummy", name="dps")
                    nc.tensor.matmul(
                        dps[:],
                        w1bT_sb[:, 0:1],
                        xT_sb[:, 1023:1024],
                        start=True,
                        stop=True,
                    )
                for hh in range(2):
                    ps = spool.tile([128, 512], FP32, tag="setup_ps", name="ps_lb")
                    nc.tensor.matmul(
                        ps[:],
                        w1bT_sb[:, hh * 128 : (hh + 1) * 128],
                        xT_sb[:, jc * 512 : (jc + 1) * 512],
                        start=True,
                        stop=True,
                    )
                    if jc == 0:
                        nc.vector.tensor_copy(
                            lbT[hh][:, jc * 512 : (jc + 1) * 512], ps[:]
                        )
                    else:
                        nc.scalar.copy(lbT[hh][:, jc * 512 : (jc + 1) * 512], ps[:])

            # ---- labT[hh] = (x_i @ W1a.T).T + b1, h on partitions ----
            labT = []
            for hh in range(2):
                t = cpool.tile([128, RPC], FP32, tag=f"labT{hh}", name=f"labT{hh}")
                labT.append(t)
                ps = spool2.tile([128, RPC], FP32, tag="setup_ps2", name="ps_la")
                nc.tensor.matmul(
                    ps[:],
                    w1aT_sb[:, hh * 128 : (hh + 1) * 128],
                    xiT_sb[:],
                    start=True,
                    stop=True,
                )
                nc.scalar.activation(
                    t[:], ps[:], AF.Identity, bias=b1c_sb[:, hh : hh + 1], scale=1.0
                )

            # ---- main loop: groups of 4 i-rows ----
            for g in range(n_groups):
                ps = [
                    mmpool.tile([128, 512], FP32, tag=f"mm{jc}", name=f"ps{jc}")
                    for jc in range(2)
                ]
                for c in range(4):
                    i = 4 * g + c
                    # single-writer relu tiles aligned to 512-col matmul chunks
                    r0a = rpool.tile([128, JA], FP32, tag="r0a", name="r0a")
                    r0b = rpool.tile([128, N - JA], FP32, tag="r0b", name="r0b")
                    r1 = rpool.tile([128, N], FP32, tag="r1", name="r1")
                    nc.scalar.activation(
                        r0a[:],
                        lbT[0][:, :JA],
                        AF.Relu,
                        bias=labT[0][:, i : i + 1],
                        scale=1.0,
                    )
                    nc.vector.tensor_scalar(
                        r0b[:],
                        lbT[0][:, JA:],
                        labT[0][:, i : i + 1],
                        0.0,
                        ALU.add,
                        ALU.max,
                    )
                    nc.vector.tensor_scalar(
                        r1[:],
                        lbT[1][:],
                        labT[1][:, i : i + 1],
                        0.0,
                        ALU.add,
                        ALU.max,
                    )
                    rhs_chunks = {
                        (0, 0): r0a[:],
                        (0, 1): r0b[:],
                        (1, 0): r1[:, :512],
                        (1, 1): r1[:, 512:],
                    }
                    for hh in range(2):
                        for jc in range(2):
                            nc.tensor.matmul(
                                ps[jc][:],
                                wst_sb(c, hh),
                                rhs_chunks[(hh, jc)],
                                start=(c == 0 and hh == 0),
                                stop=(c == 3 and hh == 1),
                            )
                # evacuate: adj row = 1 iff psum + cdiff > 0
                for jc in range(2):
                    at = apool.tile([128, 512], mybir.dt.uint8, tag="adjt", name="at")
                    nc.scalar.activation(
                        at[:], ps[jc][:], AF.Sign, bias=cbias[:], scale=1.0
                    )
                    nc.sync.dma_start(
                        out=adj8[4 * g : 4 * g + 4, jc * 512 : (jc + 1) * 512],
                        in_=at[::32, :],
                    )
    nc.compile()
    return nc


def _prep_inputs(x, W1, b1, W2, b2):
    x = np.asarray(x, dtype=np.float32)
    W1 = np.asarray(W1, dtype=np.float32)
    b1 = np.asarray(b1, dtype=np.float32)
    W2 = np.asarray(W2, dtype=np.float32)
    b2 = np.asarray(b2, dtype=np.float32)

    xT = np.ascontiguousarray(x.T)  # [D, N]
    w1aT = np.ascontiguousarray(W1[:, :D].T)  # [D, H]
    w1bT = np.ascontiguousarray(W1[:, D:].T)  # [D, H]
    b1c = np.ascontiguousarray(b1.reshape(2, 128).T)  # [128, 2]
    w = (W2[1] - W2[0]).astype(np.float32)  # [H]
    cdiff = float(np.float32(b2[1]) - np.float32(b2[0]))
    wst = np.zeros((128, 8, 128), dtype=np.float32)
    for c in range(4):
        for hh in range(2):
            wst[:, 2 * c + hh, 32 * c] = w[hh * 128 : (hh + 1) * 128]
    inA = np.concatenate([w1bT, xT], axis=1)  # [128, 1280]
    return xT, w1aT, b1c, wst, inA, cdiff


def kernel(x, W1, b1, W2, b2):
    global LAST_RESULTS
    xT, w1aT, b1c, wst, inA, cdiff = _prep_inputs(x, W1, b1, W2, b2)

    nc = build_nc(cdiff)
    in_maps = []
    for core in range(NCORES):
        xiT = xT[:, core * RPC : (core + 1) * RPC]
        inB = np.concatenate(
            [w1aT, xiT, wst.reshape(128, 1024), b1c], axis=1
        )  # [128, 1410]
        in_maps.append(dict(inA=inA, inB=np.ascontiguousarray(inB)))
    res = run_bass_kernel_spmd(nc, in_maps, list(range(NCORES)), trace=TRACE)
    LAST_RESULTS = res
    adj = np.concatenate(
        [(res.results[c]["adj8"] == 1) for c in range(NCORES)], axis=0
    ).astype(np.int32)
    np.fill_diagonal(adj, 1)
    return adj



# revision 4
# speedup vs baseline: 3.8885x; 3.8885x over previous
"""Trainium2 Bass kernel for nn_MLPBuilder (GNN message-passing edge predictor).

Math: adj[i,j] = argmax_o softmax(W2 @ relu(W1 @ cat(x_i, x_j) + b1) + b2)
            = 1  iff  w . relu(la_i + lb_j + b1) + c > 0
  where la = x @ W1[:, :D].T, lb = x @ W1[:, D:].T,
        w = W2[1] - W2[0], c = b2[1] - b2[0]   (softmax+argmax == threshold).

Sharding: rows of the N^2 pair grid, 128 i-rows per core (8 cores).

Per core (lbT/labT fp32; relu tiles + stationaries fp32r):
 - lbT[hh][h', j]  [128, 1024]: lb transposed, h on partitions (hh = h-half)
 - labT[hh][h', i] [128, 128] : la + b1 transposed (per-partition relu bias)
 - relu tiles in FP32R (11-bit-mantissa RNE rounding on write; fp32r matmuls
   run 4x faster than fp32 on the PE: 1 cycle/moving-col vs 4):
     r0  = relu(lbT[0] + labT[0][:,i])  [128,1024]  via DVE tensor_scalar (2x)
     r1a = relu(lbT[1][:, :224] + ...)  [128, 224]  via DVE
     r1b = relu(lbT[1][:, 224:] + ...)  [128, 800]  via ScalarE activation
 - h-reduction on PE, 32 i-rows per psum bank: stationary [128, 32] fp32r
   with w_half in column c -> psum row c accumulates the logit row for
   i = 32g + c. 5 matmuls per i (2 for half 0, 3 for half 1 split across
   the r1a/r1b tile boundary), 64 per bank accumulation group.
 - evacuation per group: ScalarE Sign(psum[0:32] + c) -> uint8 [32, 512],
   DMA to adj8 rows [32g, 32g+32).

Precision: only the relu outputs and w are rounded (fp32r, RNE-11);
lbT/labT/psum accumulation stay fp32. Simulated flip count vs the exact
reference: ~51 of 1M entries (rel err ~1.3e-2 < 2e-2 budget).
"""

import numpy as np

import concourse.bass as bass
import concourse.bacc as bacc
import concourse.mybir as mybir
from concourse.tile import TileContext
from concourse.bass_utils import run_bass_kernel_spmd

N, D, H = 1024, 128, 256
NCORES = 8
RPC = N // NCORES  # 128 i-rows per core
FP32 = mybir.dt.float32
FP32R = mybir.dt.float32r
GI = 32            # i-rows per psum accumulation group
NG = RPC // GI     # 4 groups
XSPL = 224         # cols of h-half 1 handled by DVE (rest on ScalarE)

# inA columns: [w1bT (256) | xT (1024)]
A_W1B, A_XT = 0, 256
# inB columns: [w1aT (256) | xiT (128) | b1c (2)]
B_W1A, B_XI, B_B1C = 0, 256, 384

TRACE = False
LAST_RESULTS = None


def build_nc(cdiff: float):
    AF = mybir.ActivationFunctionType
    ALU = mybir.AluOpType

    nc = bacc.Bacc(None, target_bir_lowering=False)
    inA = nc.declare_dram_parameter("inA", [128, 1280], FP32, isOutput=False)
    inB = nc.declare_dram_parameter("inB", [128, 386], FP32, isOutput=False)
    wst = nc.declare_dram_parameter("wst", [128, 2 * GI * GI], FP32R, isOutput=False)
    adj8 = nc.declare_dram_parameter("adj8", [RPC, N], mybir.dt.uint8, isOutput=True)

    with TileContext(nc) as tc:
        with (
            tc.tile_pool(name="const", bufs=1) as cpool,
            tc.tile_pool(name="relu", bufs=3) as rpool,
            tc.tile_pool(name="adj", bufs=2) as apool,
            tc.tile_pool(name="mm", bufs=2, space="PSUM") as mmpool,
            tc.tile_pool(name="setup_ps", bufs=2, space="PSUM") as spool,
            tc.tile_pool(name="setup_ps2", bufs=1, space="PSUM") as spool2,
            tc.tile_pool(name="dummy_ps", bufs=1, space="PSUM") as dpool,
        ):
            inA_sb = cpool.tile([128, 1280], FP32)
            # chunk 0 carries w1bT + xT[:, :512]; chunk 1 the rest of xT
            nc.sync.dma_start(out=inA_sb[:, :768], in_=inA[:, :768])
            nc.sync.dma_start(out=inA_sb[:, 768:], in_=inA[:, 768:])
            inB_sb = cpool.tile([128, 386], FP32)
            nc.sync.dma_start(out=inB_sb[:], in_=inB[:])
            wst_sb = cpool.tile([128, 2 * GI * GI], FP32R)
            nc.sync.dma_start(out=wst_sb[:], in_=wst[:])

            w1bT_sb = inA_sb[:, A_W1B : A_W1B + 256]
            xT_sb = inA_sb[:, A_XT : A_XT + 1024]
            w1aT_sb = inB_sb[:, B_W1A : B_W1A + 256]
            xiT_sb = inB_sb[:, B_XI : B_XI + 128]
            b1c_sb = inB_sb[:, B_B1C : B_B1C + 2]

            def wst_ap(c, hh):
                o = (2 * c + hh) * GI
                return wst_sb[:, o : o + GI]

            # cbias: [128,1] = cdiff, for the Sign evacuation
            cbias = cpool.tile([128, 1], FP32)
            nc.vector.memset(cbias[:], cdiff)
            # ScalarE pre-touch of inB so later ACT ops never add a DMA wait
            sct = cpool.tile([128, 1], FP32)
            nc.scalar.copy(sct[:], inB_sb[:, B_B1C : B_B1C + 1])

            # ---- lbT[hh] = (x @ W1b.T).T, h on partitions (full fp32) ----
            lbT = []
            for hh in range(2):
                t = cpool.tile([128, N], FP32, tag=f"lbT{hh}", name=f"lbT{hh}")
                lbT.append(t)
            for jc in range(2):  # jc outer: chunk-0 DMA gates jc=0 MMs only
                if jc == 1:
                    # wait-collector: absorb the chunk-1 DMA wait on PE so the
                    # real jc=1 matmuls carry only their PSUM-WAR wait
                    dps = dpool.tile([1, 1], FP32, tag="dummy", name="dps")
                    nc.tensor.matmul(
                        dps[:],
                        w1bT_sb[:, 0:1],
                        xT_sb[:, 1023:1024],
                        start=True,
                        stop=True,
                    )
                for hh in range(2):
                    ps = spool.tile([128, 512], FP32, tag="setup_ps", name="ps_lb")
                    nc.tensor.matmul(
                        ps[:],
                        w1bT_sb[:, hh * 128 : (hh + 1) * 128],
                        xT_sb[:, jc * 512 : (jc + 1) * 512],
                        start=True,
                        stop=True,
                    )
                    if jc == 0:
                        nc.vector.tensor_copy(
                            lbT[hh][:, jc * 512 : (jc + 1) * 512], ps[:]
                        )
                    else:
                        nc.scalar.copy(lbT[hh][:, jc * 512 : (jc + 1) * 512], ps[:])

            # ---- labT[hh] = (x_i @ W1a.T).T + b1, h on partitions ----
            labT = []
            for hh in range(2):
                t = cpool.tile([128, RPC], FP32, tag=f"labT{hh}", name=f"labT{hh}")
                labT.append(t)
                ps = spool2.tile([128, RPC], FP32, tag="setup_ps2", name="ps_la")
                nc.tensor.matmul(
                    ps[:],
                    w1aT_sb[:, hh * 128 : (hh + 1) * 128],
                    xiT_sb[:],
                    start=True,
                    stop=True,
                )
                nc.scalar.activation(
                    t[:], ps[:], AF.Identity, bias=b1c_sb[:, hh : hh + 1], scale=1.0
                )

            # ---- main loop: groups of GI i-rows sharing a psum bank pair ----
            for g in range(NG):
                ps0 = mmpool.tile([128, 512], FP32, tag="mm0", name="ps0")
                ps1 = mmpool.tile([128, 512], FP32, tag="mm1", name="ps1")
                if g >= 2:
                    # wait-collector: absorb the psum-WAR wait (Sign of g-2)
                    # so the first real matmuls only wait on their relu tile
                    nc.tensor.matmul(
                        ps0[0:1, 0:1],
                        w1bT_sb[:, 0:1],
                        xT_sb[:, 0:1],
                        start=True,
                        stop=True,
                        skip_group_check=True,
                    )
                for c in range(GI):
                    i = GI * g + c
                    r0 = rpool.tile([128, N], FP32R, tag="r0", name="r0")
                    r1a = rpool.tile([128, XSPL], FP32R, tag="r1a", name="r1a")
                    r1b = rpool.tile([128, N - XSPL], FP32R, tag="r1b", name="r1b")
                    nc.vector.tensor_scalar(
                        r0[:], lbT[0][:], labT[0][:, i : i + 1], 0.0, ALU.add, ALU.max
                    )
                    nc.vector.tensor_scalar(
                        r1a[:],
                        lbT[1][:, :XSPL],
                        labT[1][:, i : i + 1],
                        0.0,
                        ALU.add,
                        ALU.max,
                    )
                    nc.scalar.activation(
                        r1b[:],
                        lbT[1][:, XSPL:],
                        AF.Relu,
                        bias=labT[1][:, i : i + 1],
                        scale=1.0,
                    )
                    nc.tensor.matmul(
                        ps0[0:GI, :],
                        wst_ap(c, 0),
                        r0[:, :512],
                        start=(c == 0),
                        stop=False,
                    )
                    nc.tensor.matmul(
                        ps1[0:GI, :],
                        wst_ap(c, 0),
                        r0[:, 512:],
                        start=(c == 0),
                        stop=False,
                    )
                    nc.tensor.matmul(
                        ps0[0:GI, 0:XSPL],
                        wst_ap(c, 1),
                        r1a[:],
                        start=False,
                        stop=False,
                    )
                    nc.tensor.matmul(
                        ps0[0:GI, XSPL:512],
                        wst_ap(c, 1),
                        r1b[:, : 512 - XSPL],
                        start=False,
                        stop=(c == GI - 1),
                    )
                    nc.tensor.matmul(
                        ps1[0:GI, :],
                        wst_ap(c, 1),
                        r1b[:, 512 - XSPL :],
                        start=False,
                        stop=(c == GI - 1),
                    )
                # evacuate: adj row = 1 iff psum + cdiff > 0
                for jc, ps in ((0, ps0), (1, ps1)):
                    at = apool.tile([GI, 512], mybir.dt.uint8, tag=f"adjt{jc}", name="at")
                    nc.scalar.activation(
                        at[:], ps[0:GI, :], AF.Sign, bias=cbias[0:GI], scale=1.0
                    )
                    nc.sync.dma_start(
                        out=adj8[GI * g : GI * (g + 1), jc * 512 : (jc + 1) * 512],
                        in_=at[:],
                    )
    nc.compile()
    return nc


def _round_f32r(x):
    """Round fp32 array to the PE's fp32r grid (RNE to 11 mantissa bits)."""
    x = np.ascontiguousarray(x, dtype=np.float32)
    b = x.view(np.uint32).astype(np.uint64)
    shift = 12
    lsb = (b >> shift) & 1
    half = (1 << (shift - 1)) - 1
    r = ((b + half + lsb) >> shift) << shift
    return r.astype(np.uint32, casting="unsafe").view(np.float32)


def _prep_inputs(x, W1, b1, W2, b2):
    x = np.asarray(x, dtype=np.float32)
    W1 = np.asarray(W1, dtype=np.float32)
    b1 = np.asarray(b1, dtype=np.float32)
    W2 = np.asarray(W2, dtype=np.float32)
    b2 = np.asarray(b2, dtype=np.float32)

    xT = np.ascontiguousarray(x.T)  # [D, N]
    w1aT = np.ascontiguousarray(W1[:, :D].T)  # [D, H]
    w1bT = np.ascontiguousarray(W1[:, D:].T)  # [D, H]
    b1c = np.ascontiguousarray(b1.reshape(2, 128).T)  # [128, 2]
    w = _round_f32r(W2[1] - W2[0])  # [H], pre-rounded to the fp32r grid
    cdiff = float(np.float32(b2[1]) - np.float32(b2[0]))
    # stationaries: [128, 2*GI*GI]; (c, hh) block [128, GI] with w_half at col c
    wst = np.zeros((128, 2 * GI * GI), dtype=np.float32)
    for c in range(GI):
        for hh in range(2):
            wst[:, (2 * c + hh) * GI + c] = w[hh * 128 : (hh + 1) * 128]
    inA = np.concatenate([w1bT, xT], axis=1)  # [128, 1280]
    return xT, w1aT, b1c, wst, inA, cdiff


def kernel(x, W1, b1, W2, b2):
    global LAST_RESULTS
    xT, w1aT, b1c, wst, inA, cdiff = _prep_inputs(x, W1, b1, W2, b2)

    nc = build_nc(cdiff)
    in_maps = []
    for core in range(NCORES):
        xiT = xT[:, core * RPC : (core + 1) * RPC]
        inB = np.concatenate([w1aT, xiT, b1c], axis=1)  # [128, 386]
        in_maps.append(
            dict(inA=inA, inB=np.ascontiguousarray(inB), wst=wst)
        )
    res = run_bass_kernel_spmd(nc, in_maps, list(range(NCORES)), trace=TRACE)
    LAST_RESULTS = res
    adj = np.concatenate(
        [(res.results[c]["adj8"] == 1) for c in range(NCORES)], axis=0
    ).astype(np.int32)
    np.fill_diagonal(adj, 1)
    return adj
